# revision 5
# baseline (speedup 1.0000x reference)
"""Trainium2 Bass kernel for DipolePredictorE3NN.

Reference computation (per batch b of B=1024, over N=4096 nodes):
    s      = feats[..., :5] @ w_path0                      # scalar gate
    tp     = C01 * s * edge + C11*w_path1[0] * cross(feats[...,5:8], edge)
    g      = tp.mean(nodes)                                # [B, 3]
    out    = relu(g @ W1 + b1) @ W2 + b2                   # [B, 3]

Strategy: data-parallel over batch, 8 cores x 128 batches. On each core,
partition dim = local batch (exactly 128), free dim = nodes. w_path0 and
w_path1 are scalars at kernel-build time, so they are baked into the
instruction stream as immediates with all path constants pre-folded:
  - ScalarE: tmp_u = (C01/N * w0_u) * x_u   (5 activation muls per tile)
  - Pool:    s += tmp_u                      (4 tensor_adds per tile)
  - DVE:     9 tensor_tensor_reduce per tile, all chaining into 3
             accumulator columns via the reduce seed:
               acc_k += sum_n s*e_k                      (scale=1)
               acc_k += (+C11*w1/N) * sum_n v_a*e_b      (cross +)
               acc_k += (-C11*w1/N) * sum_n v_b*e_a      (cross -)
The accumulator [128, 0:3] IS g; column 3 is preset to 1.0 so the PE
transpose yields [g|1]T and the first matmul folds the b1 bias into its
contraction. MLP on PE in transposed form:
  hT = relu(W1b^T(k,m) . gT(k,n)),  outT = W2^T . hT + b2
Per-core output outT [3, 128]; the host concatenates and transposes.
"""

import sys

if "/opt/trn_rl_repo" not in sys.path:
    sys.path.insert(0, "/opt/trn_rl_repo")

import numpy as np

C01 = float(np.sqrt(0.5) / np.sqrt(3.0))
C11 = float(np.sqrt(0.5) / np.sqrt(6.0))

B, N = 1024, 4096
NCORES = 8
BL = B // NCORES  # 128 local batches = partition count
T = 1024  # nodes per tile
NTILES = N // T

_CACHED = {}


def _build(w0_vals, w1_val):
    import concourse.bacc as bacc
    import concourse.mybir as mybir
    from concourse import tile
    from concourse.masks import make_identity

    f32 = mybir.dt.float32
    Alu = mybir.AluOpType
    Act = mybir.ActivationFunctionType

    w0s = [float(w) * C01 / float(N) for w in w0_vals]  # pre-scaled gate weights
    c2 = float(w1_val) * C11 / float(N)  # cross-product coefficient

    nc = bacc.Bacc("TRN2", debug=False)

    feats = nc.dram_tensor("feats", [BL, N * 8], f32, kind="ExternalInput").ap()
    edge = nc.dram_tensor("edge", [BL, N * 3], f32, kind="ExternalInput").ap()
    W1 = nc.dram_tensor("W1", [3, 128], f32, kind="ExternalInput").ap()
    b1 = nc.dram_tensor("b1", [1, 128], f32, kind="ExternalInput").ap()
    W2 = nc.dram_tensor("W2", [128, 3], f32, kind="ExternalInput").ap()
    b2 = nc.dram_tensor("b2", [3, 1], f32, kind="ExternalInput").ap()
    outT = nc.dram_tensor("outT", [3, BL], f32, kind="ExternalOutput").ap()

    # cross product: (v x e)_k = v_a*e_b - v_b*e_a with (a,b) = (k+1, k+2) mod 3
    CROSS = [((k + 1) % 3, (k + 2) % 3) for k in range(3)]

    with tile.TileContext(nc) as tc:
        with (
            tc.tile_pool(name="consts", bufs=1) as consts,
            tc.tile_pool(name="state", bufs=1) as state,
            tc.tile_pool(name="io", bufs=3) as io,
            tc.tile_pool(name="sw", bufs=2) as sw,
            tc.tile_pool(name="psum", bufs=1, space="PSUM") as psum,
        ):
            # acc[:, 0:3] holds g; col 3 = 1.0 feeds the bias fold
            acc = state.tile([128, 4], f32)
            nc.vector.memset(acc[:, 3:4], 1.0)

            # per-(tile, term) partial sums: col t*9 + 3k + {0: s*e_k,
            # 1: +cross, 2: -cross}; summed into acc at the end
            pcol = state.tile([128, NTILES * 9], f32)

            dummy = state.tile([128, T], f32)

            for t in range(NTILES):
                ftile = io.tile([128, T * 8], f32, tag="f")
                nc.sync.dma_start(
                    out=ftile[:], in_=feats[:, t * T * 8 : (t + 1) * T * 8]
                )
                etile = io.tile([128, T * 3], f32, tag="e")
                nc.sync.dma_start(
                    out=etile[:], in_=edge[:, t * T * 3 : (t + 1) * T * 3]
                )

                x = [ftile[:, u :: 8] for u in range(8)]  # noqa: E203
                e = [etile[:, k :: 3] for k in range(3)]  # noqa: E203

                # s = sum_u (C01/N * w0_u) * x_u : ScalarE muls + Pool adds
                s_buf = sw.tile([128, T], f32, tag="s")
                tmps = [
                    sw.tile([128, T], f32, tag=f"tmp{u % 2}", name=f"tmp{u % 2}")
                    for u in range(2)
                ]
                nc.scalar.mul(s_buf[:], x[0], w0s[0])
                for u in range(1, 5):
                    tmp = tmps[u % 2]
                    nc.scalar.mul(tmp[:], x[u], w0s[u])
                    nc.gpsimd.tensor_add(s_buf[:], s_buf[:], tmp[:])

                # DVE: fused (in0*scale)*in1 multiply + free-axis sum
                for k in range(3):
                    a, b = CROSS[k]
                    base = t * 9 + 3 * k
                    nc.vector.affine_mul_reduce(
                        out=dummy[:], accum_out=pcol[:, base : base + 1],
                        in0=s_buf[:], in1=e[k], scale=1.0, bias=0.0,
                    )
                    nc.vector.affine_mul_reduce(
                        out=dummy[:], accum_out=pcol[:, base + 1 : base + 2],
                        in0=x[5 + a], in1=e[b], scale=c2, bias=0.0,
                    )
                    nc.vector.affine_mul_reduce(
                        out=dummy[:], accum_out=pcol[:, base + 2 : base + 3],
                        in0=x[5 + b], in1=e[a], scale=-c2, bias=0.0,
                    )

            # --- fold partials: acc[:, k] = sum over tiles and terms ---
            pcol3 = pcol[:].rearrange("p (t j) -> p t j", j=9)
            for k in range(3):
                nc.vector.tensor_reduce(
                    out=acc[:, k : k + 1], in_=pcol3[:, :, 3 * k : 3 * k + 3],
                    axis=mybir.AxisListType.XY, op=Alu.add,
                )

            # --- gT = transpose([g|1]): [128, 4] -> [4, 128] via PE ---
            identity = consts.tile([128, 128], f32)
            make_identity(nc, identity[:])
            gT_ps = psum.tile([4, 128], f32)
            nc.tensor.transpose(gT_ps[:], acc[:], identity[:])
            gT = state.tile([4, 128], f32)
            nc.scalar.copy(gT[:], gT_ps[:])

            # --- hT = relu(W1b^T(k,m) contracted with gT(k,n)) ---
            w1b_s = consts.tile([4, 128], f32)
            nc.sync.dma_start(out=w1b_s[0:3, :], in_=W1)
            nc.sync.dma_start(out=w1b_s[3:4, :], in_=b1)
            h_ps = psum.tile([128, 128], f32)
            nc.tensor.matmul(h_ps[:], lhsT=w1b_s[:], rhs=gT[:], start=True, stop=True)
            hT = state.tile([128, 128], f32)
            nc.scalar.activation(hT[:], h_ps[:], Act.Relu)

            # --- outT = W2^T . hT + b2 ---
            w2_s = consts.tile([128, 3], f32)
            nc.sync.dma_start(out=w2_s[:], in_=W2)
            b2_s = consts.tile([3, 1], f32)
            nc.sync.dma_start(out=b2_s[:], in_=b2)
            o_ps = psum.tile([3, 128], f32)
            nc.tensor.matmul(o_ps[:], lhsT=w2_s[:], rhs=hT[:], start=True, stop=True)
            oT = state.tile([3, 128], f32)
            nc.scalar.activation(oT[:], o_ps[:], Act.Identity, bias=b2_s[:])
            nc.sync.dma_start(out=outT, in_=oT[:])

    nc.finalize()
    return nc


def _get_nc(w_path0, w_path1):
    key = (
        np.asarray(w_path0, np.float32).tobytes(),
        np.asarray(w_path1, np.float32).tobytes(),
    )
    if _CACHED.get("key") != key:
        _CACHED["nc"] = _build(
            np.asarray(w_path0, np.float32).reshape(5),
            float(np.asarray(w_path1, np.float32).reshape(1)[0]),
        )
        _CACHED["key"] = key
    return _CACHED["nc"]


def _in_maps(feats, edge_attr, W1, b1, W2, b2):
    f32 = np.float32
    W1m = np.ascontiguousarray(W1, f32).reshape(3, 128)
    b1m = np.ascontiguousarray(b1, f32).reshape(1, 128)
    W2m = np.ascontiguousarray(W2, f32).reshape(128, 3)
    b2m = np.ascontiguousarray(b2, f32).reshape(3, 1)
    maps = []
    for c in range(NCORES):
        sl = slice(c * BL, (c + 1) * BL)
        maps.append(
            {
                "feats": np.ascontiguousarray(feats[sl], f32).reshape(BL, N * 8),
                "edge": np.ascontiguousarray(edge_attr[sl], f32).reshape(BL, N * 3),
                "W1": W1m,
                "b1": b1m,
                "W2": W2m,
                "b2": b2m,
            }
        )
    return maps


def run(inputs, trace=False, tmpdir=None):
    """Run on 8 cores; returns (out [B,3], BassKernelResults)."""
    from concourse import bass_utils

    nc = _get_nc(inputs["w_path0"], inputs["w_path1"])
    maps = _in_maps(
        inputs["feats"], inputs["edge_attr"],
        inputs["W1"], inputs["b1"], inputs["W2"], inputs["b2"],
    )
    kw = {}
    if trace:
        kw.update(trace=True, tmpdir=tmpdir)
    res = bass_utils.run_bass_kernel_spmd(
        nc, maps, core_ids=list(range(NCORES)), **kw
    )
    outT_full = np.concatenate([r["outT"] for r in res.results], axis=1)  # [3, B]
    return np.ascontiguousarray(outT_full.T), res


def kernel(feats, edge_attr, w_path0, w_path1, W1, b1, W2, b2):
    out, _ = run(
        dict(
            feats=feats, edge_attr=edge_attr, w_path0=w_path0, w_path1=w_path1,
            W1=W1, b1=b1, W2=W2, b2=b2,
        )
    )
    return out


# revision 6
# speedup vs baseline: 1.2772x; 1.2772x over previous
"""Trainium2 Bass kernel for DipolePredictorE3NN.

Reference computation (per batch b of B=1024, over N=4096 nodes):
    s      = feats[..., :5] @ w_path0                      # scalar gate
    tp     = C01 * s * edge + C11*w_path1[0] * cross(feats[...,5:8], edge)
    g      = tp.mean(nodes)                                # [B, 3]
    out    = relu(g @ W1 + b1) @ W2 + b2                   # [B, 3]

Strategy: data-parallel over batch, 8 cores x 128 batches. On each core,
partition dim = local batch (exactly 128), free dim = nodes. The host
pre-transposes each core's shard to channel-planar [BL, C, N] so every
on-chip operand is a dense unit-stride [128, T] plane (strided SBUF
reads cost 2-4 cyc/elem on DVE/ACT/Pool — measured 1.9-4.0us vs 0.6-1.3us
per op). w_path0/w_path1 are baked as immediates with path constants
pre-folded. Per node-tile:
  - ScalarE: tmp_u = (C01/N * w0_u) * x_u       (5 dense muls)
  - s accumulation adds alternate Pool / DVE by tile parity
  - DVE: 9 affine_mul_reduce (custom DVE op): out=(in0*scale)*in1,
    accum_out = sum(out), writing per-(tile,term) partial columns:
      sum_n s*e_k  (scale=1), +-C11*w1/N * sum_n v_a*e_b  (cross terms)
Partials fold with one strided tensor_reduce per k. The resulting g
[128, 0:3] plus a ones column feeds a PE transpose, then the MLP runs on
the PE in transposed form (b1 folded into the contraction, b2 via the
activation bias): hT = relu(W1b^T . [g|1]T), outT = W2^T . hT + b2.
Per-core output outT [3, 128]; the host concatenates and transposes.
"""

import sys

if "/opt/trn_rl_repo" not in sys.path:
    sys.path.insert(0, "/opt/trn_rl_repo")

import numpy as np

C01 = float(np.sqrt(0.5) / np.sqrt(3.0))
C11 = float(np.sqrt(0.5) / np.sqrt(6.0))

B, N = 1024, 4096
NCORES = 8
BL = B // NCORES  # 128 local batches = partition count
T = 1024  # nodes per tile
NTILES = N // T

_CACHED = {}


def _build(w0_vals, w1_val):
    import concourse.bacc as bacc
    import concourse.mybir as mybir
    from concourse import tile
    from concourse.masks import make_identity

    f32 = mybir.dt.float32
    Alu = mybir.AluOpType
    Act = mybir.ActivationFunctionType

    w0s = [float(w) * C01 / float(N) for w in w0_vals]  # pre-scaled gate weights
    c2 = float(w1_val) * C11 / float(N)  # cross-product coefficient

    nc = bacc.Bacc("TRN2", debug=False)

    feats = nc.dram_tensor("feats", [BL, 8, N], f32, kind="ExternalInput").ap()
    edge = nc.dram_tensor("edge", [BL, 3, N], f32, kind="ExternalInput").ap()
    W1 = nc.dram_tensor("W1", [3, 128], f32, kind="ExternalInput").ap()
    b1 = nc.dram_tensor("b1", [1, 128], f32, kind="ExternalInput").ap()
    W2 = nc.dram_tensor("W2", [128, 3], f32, kind="ExternalInput").ap()
    b2 = nc.dram_tensor("b2", [3, 1], f32, kind="ExternalInput").ap()
    outT = nc.dram_tensor("outT", [3, BL], f32, kind="ExternalOutput").ap()

    # cross product: (v x e)_k = v_a*e_b - v_b*e_a with (a,b) = (k+1, k+2) mod 3
    CROSS = [((k + 1) % 3, (k + 2) % 3) for k in range(3)]

    with tile.TileContext(nc) as tc:
        with (
            tc.tile_pool(name="consts", bufs=1) as consts,
            tc.tile_pool(name="state", bufs=1) as state,
            tc.tile_pool(name="io", bufs=3) as io,
            tc.tile_pool(name="sw", bufs=2) as sw,
            tc.tile_pool(name="psum", bufs=1, space="PSUM") as psum,
        ):
            # acc[:, 0:3] holds g; col 3 = 1.0 feeds the bias fold
            acc = state.tile([128, 4], f32)
            nc.vector.memset(acc[:, 3:4], 1.0)

            # per-(tile, term) partial sums: col t*9 + 3k + {0: s*e_k,
            # 1: +cross, 2: -cross}; summed into acc at the end
            pcol = state.tile([128, NTILES * 9], f32)

            dummy = state.tile([128, T], f32)

            for t in range(NTILES):
                ftile = io.tile([128, 8 * T], f32, tag="f")
                f3 = ftile[:].rearrange("p (c n) -> p c n", c=8)
                nc.sync.dma_start(out=f3, in_=feats[:, :, t * T : (t + 1) * T])
                etile = io.tile([128, 3 * T], f32, tag="e")
                e3 = etile[:].rearrange("p (c n) -> p c n", c=3)
                nc.sync.dma_start(out=e3, in_=edge[:, :, t * T : (t + 1) * T])

                x = [ftile[:, u * T : (u + 1) * T] for u in range(8)]
                e = [etile[:, k * T : (k + 1) * T] for k in range(3)]

                # s = sum_u (C01/N * w0_u) * x_u : ScalarE muls; accumulate
                # adds alternate Pool / DVE so neither engine gates
                add_eng = nc.gpsimd if t % 2 == 0 else nc.vector
                s_buf = sw.tile([128, T], f32, tag="s")
                tmps = [
                    sw.tile([128, T], f32, tag=f"tmp{u % 2}", name=f"tmp{u % 2}")
                    for u in range(2)
                ]
                nc.scalar.mul(s_buf[:], x[0], w0s[0])
                for u in range(1, 5):
                    tmp = tmps[u % 2]
                    nc.scalar.mul(tmp[:], x[u], w0s[u])
                    add_eng.tensor_add(s_buf[:], s_buf[:], tmp[:])

                # DVE: fused (in0*scale)*in1 multiply + free-axis sum
                for k in range(3):
                    a, b = CROSS[k]
                    base = t * 9 + 3 * k
                    nc.vector.affine_mul_reduce(
                        out=dummy[:], accum_out=pcol[:, base : base + 1],
                        in0=s_buf[:], in1=e[k], scale=1.0, bias=0.0,
                    )
                    nc.vector.affine_mul_reduce(
                        out=dummy[:], accum_out=pcol[:, base + 1 : base + 2],
                        in0=x[5 + a], in1=e[b], scale=c2, bias=0.0,
                    )
                    nc.vector.affine_mul_reduce(
                        out=dummy[:], accum_out=pcol[:, base + 2 : base + 3],
                        in0=x[5 + b], in1=e[a], scale=-c2, bias=0.0,
                    )

            # --- fold partials: acc[:, k] = sum over tiles and terms ---
            pcol3 = pcol[:].rearrange("p (t j) -> p t j", j=9)
            for k in range(3):
                nc.vector.tensor_reduce(
                    out=acc[:, k : k + 1], in_=pcol3[:, :, 3 * k : 3 * k + 3],
                    axis=mybir.AxisListType.XY, op=Alu.add,
                )

            # --- gT = transpose([g|1]): [128, 4] -> [4, 128] via PE ---
            identity = consts.tile([128, 128], f32)
            make_identity(nc, identity[:])
            gT_ps = psum.tile([4, 128], f32)
            nc.tensor.transpose(gT_ps[:], acc[:], identity[:])
            gT = state.tile([4, 128], f32)
            nc.scalar.copy(gT[:], gT_ps[:])

            # --- hT = relu(W1b^T(k,m) contracted with gT(k,n)) ---
            w1b_s = consts.tile([4, 128], f32)
            nc.sync.dma_start(out=w1b_s[0:3, :], in_=W1)
            nc.sync.dma_start(out=w1b_s[3:4, :], in_=b1)
            h_ps = psum.tile([128, 128], f32)
            nc.tensor.matmul(h_ps[:], lhsT=w1b_s[:], rhs=gT[:], start=True, stop=True)
            hT = state.tile([128, 128], f32)
            nc.scalar.activation(hT[:], h_ps[:], Act.Relu)

            # --- outT = W2^T . hT + b2 ---
            w2_s = consts.tile([128, 3], f32)
            nc.sync.dma_start(out=w2_s[:], in_=W2)
            b2_s = consts.tile([3, 1], f32)
            nc.sync.dma_start(out=b2_s[:], in_=b2)
            o_ps = psum.tile([3, 128], f32)
            nc.tensor.matmul(o_ps[:], lhsT=w2_s[:], rhs=hT[:], start=True, stop=True)
            oT = state.tile([3, 128], f32)
            nc.scalar.activation(oT[:], o_ps[:], Act.Identity, bias=b2_s[:])
            nc.sync.dma_start(out=outT, in_=oT[:])

    nc.finalize()
    return nc


def _get_nc(w_path0, w_path1):
    key = (
        np.asarray(w_path0, np.float32).tobytes(),
        np.asarray(w_path1, np.float32).tobytes(),
    )
    if _CACHED.get("key") != key:
        _CACHED["nc"] = _build(
            np.asarray(w_path0, np.float32).reshape(5),
            float(np.asarray(w_path1, np.float32).reshape(1)[0]),
        )
        _CACHED["key"] = key
    return _CACHED["nc"]


def _in_maps(feats, edge_attr, W1, b1, W2, b2):
    f32 = np.float32
    W1m = np.ascontiguousarray(W1, f32).reshape(3, 128)
    b1m = np.ascontiguousarray(b1, f32).reshape(1, 128)
    W2m = np.ascontiguousarray(W2, f32).reshape(128, 3)
    b2m = np.ascontiguousarray(b2, f32).reshape(3, 1)
    feats = np.asarray(feats, f32)
    edge_attr = np.asarray(edge_attr, f32)
    maps = []
    for c in range(NCORES):
        sl = slice(c * BL, (c + 1) * BL)
        maps.append(
            {
                # channel-planar [BL, C, N] so on-chip planes are unit-stride
                "feats": np.ascontiguousarray(feats[sl].transpose(0, 2, 1)),
                "edge": np.ascontiguousarray(edge_attr[sl].transpose(0, 2, 1)),
                "W1": W1m,
                "b1": b1m,
                "W2": W2m,
                "b2": b2m,
            }
        )
    return maps


def run(inputs, trace=False, tmpdir=None):
    """Run on 8 cores; returns (out [B,3], BassKernelResults)."""
    from concourse import bass_utils

    nc = _get_nc(inputs["w_path0"], inputs["w_path1"])
    maps = _in_maps(
        inputs["feats"], inputs["edge_attr"],
        inputs["W1"], inputs["b1"], inputs["W2"], inputs["b2"],
    )
    kw = {}
    if trace:
        kw.update(trace=True, tmpdir=tmpdir)
    res = bass_utils.run_bass_kernel_spmd(
        nc, maps, core_ids=list(range(NCORES)), **kw
    )
    outT_full = np.concatenate([r["outT"] for r in res.results], axis=1)  # [3, B]
    return np.ascontiguousarray(outT_full.T), res


def kernel(feats, edge_attr, w_path0, w_path1, W1, b1, W2, b2):
    out, _ = run(
        dict(
            feats=feats, edge_attr=edge_attr, w_path0=w_path0, w_path1=w_path1,
            W1=W1, b1=b1, W2=W2, b2=b2,
        )
    )
    return out
